# revision 11
# baseline (speedup 1.0000x reference)
"""Trainium2 Bass kernel for nn_Block_84258668413379 (dense transformer block).

Strategy: data-parallel over batch B=16 across 8 NeuronCores (2 batches/core),
no collectives.  All matmuls in bf16 (f32 PSUM accumulation); layernorm and
softmax statistics in f32.  Transposed dataflow avoids attention-probability
transposes entirely:

  x --LN1--> xh --PE transpose--> h^T
  Q^T,K^T = Wqk'^T.T @ h^T        (output-transposed, bias fused in ACT copy)
  V       = h^T.T @ Wv'^T         (natural orientation)
  S^T     = K_h^T.T @ Q_h^T       [keys x queries]
  P^T     = exp(S^T) * exp(bias^T)   (bias exponential precomputed host-side:
                                      ACT exp reads PSUM directly, DVE multiply
                                      runs SBUF->SBUF in bf16)
  O^T     = [V_h | 1].T @ P^T     (extra column = softmax denominator)
  O_norm^T= O^T * bcast(1/s)      (GpSimd partition_broadcast + DVE mult)
  y       = O_norm^T.T @ Wp'^T ; x1 = x + y        (gamma1 folded into Wp')
  x1 --LN2--> xh2 --transpose--> h2^T (branch-permuted columns)
  g^T     = gelu(fc1'^T.T @ h2^T)  (sigmoid-form gelu: ACT sigmoid + DVE fuse)
  y2      = g^T.T @ fc2'^T ; out = x1 + y2         (gamma2 folded into fc2')

All LN affine params and additive biases are folded into weights host-side;
bias inputs that are all-zero emit no device instructions.
"""

import numpy as np

import concourse.bass as bass
import concourse.bacc as bacc
import concourse.mybir as mybir
from concourse import tile
from concourse.bass_utils import run_bass_kernel_spmd

# Problem constants (hardcoded per spec)
B, N, C, H = 16, 512, 768, 12
D = C // H                      # 64
SPLIT = 256
TABLE = 1024
HID = 4 * C                     # 3072
SCALE = D ** -0.5
EPS = 1e-5
NCORES = 8
BL = B // NCORES                # 2 batches per core
T = BL * N                      # 1024 tokens per core
NT = T // 128                   # 8 token tiles
CC = C // 128                   # 6 contraction chunks
OC = HID // 128                 # 24 hidden chunks
GELU_A = 1.702

F32 = mybir.dt.float32
BF16 = mybir.dt.bfloat16
NP_BF16 = mybir.dt.np(BF16)

# token tile -> h2T column block (group branch-t tokens, then branch-f)
H2T_PERM = [0, 1, 4, 5, 2, 3, 6, 7]
# fc2 token-chunk (branch-major) -> x tile index
FC2_TILES = [[0, 1, 4, 5], [2, 3, 6, 7]]  # [branch][tc]


def _f32(a):
    return np.ascontiguousarray(np.asarray(a, dtype=np.float32))


def prep_inputs(x, mask, relative_position_index, W_qkv, W_proj, b_proj, rel_table,
                ln1_g, ln1_b, gamma1, gamma2,
                ln2t_g, ln2t_b, fc1t_W, fc1t_b, fc2t_W, fc2t_b,
                ln2f_g, ln2f_b, fc1f_W, fc1f_b, fc2f_W, fc2f_b):
    """Host-side folding / layout prep. Returns (shared, per_core, flags)."""
    x = _f32(x); W_qkv = _f32(W_qkv); W_proj = _f32(W_proj); b_proj = _f32(b_proj)
    rel_table = _f32(rel_table)
    ln1_g = _f32(ln1_g); ln1_b = _f32(ln1_b)
    gamma1 = _f32(gamma1); gamma2 = _f32(gamma2)
    mask = np.asarray(mask)
    rpi = np.asarray(relative_position_index).astype(np.int64)

    # ---- QK weights (ln1 gain + q-scale folded) -> wqk[p, m, cc, j] ----
    Wqk = W_qkv[:2 * C] * ln1_g[None, :]
    Wqk[:C] *= SCALE
    WqkT = Wqk.T.copy()                                   # [C, 2C]
    wqk = WqkT.reshape(CC, 128, 12, 128).transpose(1, 2, 0, 3)  # [128,12,6,128]
    wqk = np.ascontiguousarray(wqk.astype(NP_BF16))

    bias_qk = (W_qkv[:2 * C] @ ln1_b)
    bias_qk[:C] *= SCALE
    bqk = np.ascontiguousarray(bias_qk.reshape(12, 128).T.astype(np.float32))  # [128,12]

    # ---- V weights -> wv[p, cc, j] ----
    Wv = W_qkv[2 * C:] * ln1_g[None, :]
    WvT = Wv.T.copy()                                     # [C, C]
    wv = np.ascontiguousarray(
        WvT.reshape(CC, 128, C).transpose(1, 0, 2).astype(NP_BF16))  # [128,6,768]
    bias_v = W_qkv[2 * C:] @ ln1_b                        # [C]
    vb = np.ascontiguousarray(bias_v.reshape(CC, 128).T.astype(np.float32))  # [128,6]
    use_vbias = bool(np.any(bias_v != 0.0))

    # ---- proj weights (gamma1 folded) -> wp[p, cc, j] ----
    Wp = gamma1[:, None] * W_proj                         # [C, C] (out, in)
    WpT = Wp.T.copy()
    wp = np.ascontiguousarray(
        WpT.reshape(CC, 128, C).transpose(1, 0, 2).astype(NP_BF16))  # [128,6,768]
    bias_p = gamma1 * b_proj
    pb = np.ascontiguousarray(bias_p[None, :].astype(NP_BF16))       # [1, 768]
    use_pbias = bool(np.any(bias_p != 0.0))

    # ---- exp(relative position bias), S^T layout: expB[h, kc, p(key), q] ----
    bias_full = rel_table[rpi]                            # [Nq, Nk, H]
    biasT = bias_full.transpose(2, 1, 0)                  # [H, Nk, Nq]
    mask_uniform = bool(mask.all())

    def _pack_expb(bt):
        e = np.exp(bt.astype(np.float64)).astype(np.float32)
        return np.ascontiguousarray(
            e.reshape(H, 4, 128, N).astype(NP_BF16))      # [12,4,128,512]

    per_core_expb = None
    if mask_uniform:
        expb = _pack_expb(biasT)
    else:
        mterm = np.where(mask, 0.0, -np.inf).astype(np.float32)      # [B, Nk]
        per_core_expb = []
        for c in range(NCORES):
            packs = []
            for b in range(BL):
                bt = biasT + mterm[c * BL + b][None, :, None]
                packs.append(_pack_expb(bt))
            per_core_expb.append(np.stack(packs))         # [BL,12,4,128,512]
        expb = per_core_expb[0]

    # ---- MLP weights per branch ----
    fc1w = np.empty((2, OC, 128, CC, 128), dtype=NP_BF16)
    fc2w = np.empty((2, OC, 128, C), dtype=NP_BF16)
    bfc1 = np.zeros((128, 2, OC), dtype=np.float32)
    bfc2 = np.zeros((1, 2, C), dtype=NP_BF16)
    use_fc1bias = False
    use_fc2bias = False
    for br, (g2, b2, w1, b1, w2, bz2) in enumerate([
            (ln2t_g, ln2t_b, fc1t_W, fc1t_b, fc2t_W, fc2t_b),
            (ln2f_g, ln2f_b, fc1f_W, fc1f_b, fc2f_W, fc2f_b)]):
        g2 = _f32(g2); b2 = _f32(b2); w1 = _f32(w1); b1 = _f32(b1)
        w2 = _f32(w2); bz2 = _f32(bz2)
        W1 = w1 * g2[None, :]                             # [HID, C]
        W1T = W1.T.copy()                                 # [C, HID]
        fc1w[br] = W1T.reshape(CC, 128, OC, 128).transpose(2, 1, 0, 3).astype(NP_BF16)
        b1f = w1 @ b2 + b1                                # [HID]
        bfc1[:, br, :] = b1f.reshape(OC, 128).T
        use_fc1bias = use_fc1bias or bool(np.any(b1f != 0.0))

        W2 = gamma2[:, None] * w2                         # [C, HID]
        W2T = W2.T.copy()                                 # [HID, C]
        fc2w[br] = W2T.reshape(OC, 128, C).astype(NP_BF16)
        b2f = gamma2 * bz2
        bfc2[0, br, :] = b2f.astype(NP_BF16)
        use_fc2bias = use_fc2bias or bool(np.any(b2f != 0.0))

    shared = dict(
        wqk=wqk, wv=wv, wp=wp, fc1w=np.ascontiguousarray(fc1w),
        fc2w=np.ascontiguousarray(fc2w), expb=expb, bqk=bqk,
        ident=np.ascontiguousarray(np.eye(128, dtype=NP_BF16)),
        vb=vb, pb=pb, bfc1=np.ascontiguousarray(bfc1), bfc2=bfc2,
        bfc1s=np.ascontiguousarray(GELU_A * bfc1),
    )
    per_core_x = [np.ascontiguousarray(x[c * BL:(c + 1) * BL].reshape(T, C))
                  for c in range(NCORES)]
    flags = dict(mask_uniform=mask_uniform, use_vbias=use_vbias,
                 use_pbias=use_pbias, use_fc1bias=use_fc1bias,
                 use_fc2bias=use_fc2bias)
    return shared, (per_core_x, per_core_expb), flags


def build_program(flags):
    """Emit the per-core Bass/Tile program. Same program runs SPMD on 8 cores."""
    mask_uniform = flags["mask_uniform"]
    nc = bacc.Bacc("TRN2", target_bir_lowering=False, debug=False,
                   num_devices=NCORES)

    x_d = nc.dram_tensor("x", [T, C], F32, kind="ExternalInput")
    wqk_d = nc.dram_tensor("wqk", [128, 12, CC, 128], BF16, kind="ExternalInput")
    wv_d = nc.dram_tensor("wv", [128, CC, C], BF16, kind="ExternalInput")
    wp_d = nc.dram_tensor("wp", [128, CC, C], BF16, kind="ExternalInput")
    fc1w_d = nc.dram_tensor("fc1w", [2, OC, 128, CC, 128], BF16, kind="ExternalInput")
    fc2w_d = nc.dram_tensor("fc2w", [2, OC, 128, C], BF16, kind="ExternalInput")
    if mask_uniform:
        expb_d = nc.dram_tensor("expb", [H, 4, 128, N], BF16, kind="ExternalInput")
    else:
        expb_d = nc.dram_tensor("expb", [BL, H, 4, 128, N], BF16,
                                kind="ExternalInput")
    bqk_d = nc.dram_tensor("bqk", [128, 12], F32, kind="ExternalInput")
    ident_d = nc.dram_tensor("ident", [128, 128], BF16, kind="ExternalInput")
    vb_d = nc.dram_tensor("vb", [128, CC], F32, kind="ExternalInput")
    pb_d = nc.dram_tensor("pb", [1, C], BF16, kind="ExternalInput")
    bfc1_d = nc.dram_tensor("bfc1", [128, 2, OC], F32, kind="ExternalInput")
    bfc1s_d = nc.dram_tensor("bfc1s", [128, 2, OC], F32, kind="ExternalInput")
    bfc2_d = nc.dram_tensor("bfc2", [1, 2, C], BF16, kind="ExternalInput")
    out_d = nc.dram_tensor("out", [T, C], F32, kind="ExternalOutput")

    AL = mybir.AluOpType
    AF = mybir.ActivationFunctionType

    with tile.TileContext(nc) as tc:
        with (
            tc.tile_pool(name="const", bufs=1) as cpool,
            tc.tile_pool(name="res", bufs=1) as rpool,
            tc.tile_pool(name="work", bufs=3) as wpool,
            tc.tile_pool(name="wt", bufs=3) as wtpool,
            tc.tile_pool(name="bias2", bufs=2) as bpool,
            tc.tile_pool(name="stat", bufs=4) as spool,
            tc.tile_pool(name="ps", bufs=3, space="PSUM") as ps,       # st: 3 banks
            tc.tile_pool(name="ps2", bufs=2, space="PSUM") as ps2,     # tr,st256: 4
            tc.tile_pool(name="ps3", bufs=1, space="PSUM") as ps3,     # ot: 1 bank
        ):
            # ---------------- constants ----------------
            wqk_sb = cpool.tile([128, 12, CC, 128], BF16, tag="wqk")
            nc.sync.dma_start(wqk_sb[:], wqk_d[:])
            wv_sb = cpool.tile([128, CC, C], BF16, tag="wv")
            nc.sync.dma_start(wv_sb[:], wv_d[:])
            wp_sb = cpool.tile([128, CC, C], BF16, tag="wp")
            nc.sync.dma_start(wp_sb[:], wp_d[:])
            bqk_sb = cpool.tile([128, 12], F32, tag="bqk")
            nc.sync.dma_start(bqk_sb[:], bqk_d[:])
            ident_sb = cpool.tile([128, 128], BF16, tag="ident")
            nc.sync.dma_start(ident_sb[:], ident_d[:])
            onescol = cpool.tile([128, 1], BF16, tag="onescol")
            nc.vector.memset(onescol[:], 1.0)
            eps_sb = cpool.tile([128, 1], F32, tag="eps")
            nc.vector.memset(eps_sb[:], EPS)
            if flags["use_vbias"]:
                vb_sb = cpool.tile([128, CC], F32, tag="vb")
                nc.sync.dma_start(vb_sb[:], vb_d[:])
            if flags["use_pbias"]:
                pb_sb = cpool.tile([1, C], BF16, tag="pb")
                nc.sync.dma_start(pb_sb[:], pb_d[:])
                ones1 = cpool.tile([1, 128], BF16, tag="ones1")
                nc.vector.memset(ones1[:], 1.0)
            if flags["use_fc1bias"]:
                bfc1_sb = cpool.tile([128, 2, OC], F32, tag="bfc1")
                nc.sync.dma_start(bfc1_sb[:], bfc1_d[:])
                bfc1s_sb = cpool.tile([128, 2, OC], F32, tag="bfc1s")
                nc.sync.dma_start(bfc1s_sb[:], bfc1s_d[:])
            if flags["use_fc2bias"]:
                bfc2_sb = cpool.tile([1, 2, C], BF16, tag="bfc2")
                nc.sync.dma_start(bfc2_sb[:], bfc2_d[:])
                ones1b = cpool.tile([1, 128], BF16, tag="ones1b")
                nc.vector.memset(ones1b[:], 1.0)

            # ---------------- residents ----------------
            x_sb = [rpool.tile([128, C], F32, tag=f"x{t}", name=f"x_sb{t}")
                    for t in range(NT)]
            hT = rpool.tile([128, CC, T], BF16, tag="hT")         # h1T then h2T
            qkT = rpool.tile([128, 12, T], BF16, tag="qkg")       # later reused: gT
            v65 = rpool.tile([128, NT, H, D + 1], BF16, tag="v65")
            oTn = rpool.tile([128, CC, T], BF16, tag="oTn")

            def layernorm(x_t, xh_t):
                """xh_t (bf16) = (x_t - mean)/sqrt(var+eps), stats in f32."""
                st6 = spool.tile([128, 2, 6], F32, tag="st6")
                nc.vector.bn_stats(st6[:, 0, :], x_t[:, 0:C // 2])
                nc.vector.bn_stats(st6[:, 1, :], x_t[:, C // 2:C])
                mv = spool.tile([128, 2], F32, tag="mv")
                nc.vector.bn_aggr(mv[:], st6[:])
                sd = spool.tile([128, 1], F32, tag="sd")
                nc.scalar.activation(sd[:], mv[:, 1:2], AF.Sqrt, bias=eps_sb[:])
                rs = spool.tile([128, 1], F32, tag="rs")
                nc.vector.reciprocal(rs[:], sd[:])
                nc.vector.tensor_scalar(xh_t[:], x_t[:], mv[:, 0:1], rs[:],
                                        op0=AL.subtract, op1=AL.mult)

            def transpose_into(dst, dst_col, xh_t):
                """PE-transpose xh_t [128, C] into dst[:, cc, dst_col:+128]."""
                for cc in range(CC):
                    tr = ps2.tile([128, 128], BF16, tag="tr")
                    nc.tensor.transpose(tr[:], xh_t[:, cc * 128:(cc + 1) * 128],
                                        ident_sb[:])
                    nc.vector.tensor_copy(dst[:, cc, dst_col:dst_col + 128], tr[:])

            # ---------------- phase 1+2: LN1 + h1T ----------------
            for t in range(NT):
                nc.sync.dma_start(x_sb[t][:], x_d[t * 128:(t + 1) * 128, :])
                xh = wpool.tile([128, C], BF16, tag="xh")
                layernorm(x_sb[t], xh)
                transpose_into(hT, t * 128, xh)

            # ---------------- phase 3: Q^T / K^T ----------------
            for m in range(12):
                for n in range(2):
                    pt = ps.tile([128, 512], F32, tag="st")
                    for cc in range(CC):
                        nc.tensor.matmul(
                            pt[:], wqk_sb[:, m, cc, :],
                            hT[:, cc, n * 512:(n + 1) * 512],
                            start=(cc == 0), stop=(cc == CC - 1))
                    nc.scalar.activation(qkT[:, m, n * 512:(n + 1) * 512], pt[:],
                                         AF.Identity, bias=bqk_sb[:, m:m + 1])

            # ---------------- phase 4: V ----------------
            nc.vector.memset(v65[:, :, :, D:D + 1], 1.0)   # ones column
            for t in range(NT):
                p5 = ps.tile([128, 512], F32, tag="st")
                p2 = ps2.tile([128, 256], F32, tag="st256")
                for cc in range(CC):
                    nc.tensor.matmul(p5[:], hT[:, cc, t * 128:(t + 1) * 128],
                                     wv_sb[:, cc, 0:512],
                                     start=(cc == 0), stop=(cc == CC - 1))
                for cc in range(CC):
                    nc.tensor.matmul(p2[:], hT[:, cc, t * 128:(t + 1) * 128],
                                     wv_sb[:, cc, 512:768],
                                     start=(cc == 0), stop=(cc == CC - 1))
                nc.scalar.copy(
                    v65[:, t, 0:8, 0:D],
                    p5[:].rearrange("p (h d) -> p h d", d=D))
                nc.scalar.copy(
                    v65[:, t, 8:12, 0:D],
                    p2[:].rearrange("p (h d) -> p h d", d=D))

            # ---------------- phase 5: attention ----------------
            for h in range(12):
                par = h % 2
                base = par * 64
                m_q, m_k = h // 2, 6 + h // 2
                eb_tiles = []
                if mask_uniform:
                    for kc in range(4):
                        eb = bpool.tile([128, 512], BF16, tag=f"eb{kc}",
                                        name=f"eb_{h}_{kc}")
                        nc.sync.dma_start(eb[:], expb_d[h, kc])
                        eb_tiles.append(eb)
                for b in range(BL):
                    if not mask_uniform:
                        eb_tiles = []
                        for kc in range(4):
                            eb = bpool.tile([128, 512], BF16, tag=f"eb{kc}",
                                            name=f"eb_{h}_{b}_{kc}")
                            nc.sync.dma_start(eb[:], expb_d[b, h, kc])
                            eb_tiles.append(eb)
                    qs = qkT[base:base + 64, m_q, b * 512:(b + 1) * 512]
                    pts = []
                    for kc in range(4):
                        st = ps.tile([128, 512], F32, tag="st",
                                     name=f"st_{h}_{b}_{kc}")
                        nc.tensor.matmul(
                            st[:],
                            qkT[base:base + 64, m_k,
                                b * 512 + kc * 128:b * 512 + (kc + 1) * 128],
                            qs)
                        pe = wpool.tile([128, 512], BF16, tag="pe")
                        nc.scalar.activation(pe[:], st[:], AF.Exp)
                        pt_t = bpool.tile([128, 512], BF16, tag=f"pT{kc}",
                                          name=f"pT_{h}_{b}_{kc}")
                        nc.vector.tensor_tensor(pt_t[:], pe[:], eb_tiles[kc][:],
                                                op=AL.mult)
                        pts.append(pt_t)
                    # O^T (+ softmax denominator)
                    ot = ps3.tile([128, 512], F32, tag="ot", name=f"ot_{h}_{b}")
                    if par == 0:
                        for kc in range(4):
                            nc.tensor.matmul(ot[0:65, :],
                                             v65[:, b * 4 + kc, h, 0:D + 1],
                                             pts[kc][:],
                                             start=(kc == 0), stop=(kc == 3))
                        srow = ot[64:65, :]
                        orows = ot[0:64, :]
                    else:
                        for kc in range(4):
                            nc.tensor.matmul(ot[64:128, :],
                                             v65[:, b * 4 + kc, h, 0:D],
                                             pts[kc][:],
                                             start=(kc == 0), stop=(kc == 3))
                        ss = ps2.tile([65, 512], F32, tag="tr",
                                      name=f"ss_{h}_{b}")
                        for kc in range(4):
                            nc.tensor.matmul(ss[64:65, :], onescol[:],
                                             pts[kc][:],
                                             start=(kc == 0), stop=(kc == 3))
                        srow = ss[64:65, :]
                        orows = ot[64:128, :]
                    rrec = wpool.tile([65, 512], F32, tag="rrec")
                    nc.vector.reciprocal(rrec[64:65, :], srow)
                    rbc = wpool.tile([128, 512], F32, tag="rbc")
                    nc.gpsimd.partition_broadcast(rbc[base:base + 64, :],
                                                  rrec[64:65, :], channels=64)
                    dst = oTn[base:base + 64, h // 2, b * 512:(b + 1) * 512]
                    nc.vector.tensor_tensor(dst, orows, rbc[base:base + 64, :],
                                            op=AL.mult)
                    if flags["use_vbias"]:
                        nc.vector.tensor_scalar_add(
                            dst, dst, vb_sb[base:base + 64, h // 2:h // 2 + 1])

            # ---------------- phase 6: proj + residual ----------------
            for t in range(NT):
                p5 = ps.tile([128, 512], F32, tag="st")
                p2 = ps2.tile([128, 256], F32, tag="st256")
                nsteps = CC + (1 if flags["use_pbias"] else 0)
                for cc in range(CC):
                    nc.tensor.matmul(p5[:], oTn[:, cc, t * 128:(t + 1) * 128],
                                     wp_sb[:, cc, 0:512],
                                     start=(cc == 0), stop=(cc == nsteps - 1))
                for cc in range(CC):
                    nc.tensor.matmul(p2[:], oTn[:, cc, t * 128:(t + 1) * 128],
                                     wp_sb[:, cc, 512:768],
                                     start=(cc == 0), stop=(cc == nsteps - 1))
                if flags["use_pbias"]:
                    nc.tensor.matmul(p5[:], ones1[:], pb_sb[:, 0:512],
                                     start=False, stop=True)
                    nc.tensor.matmul(p2[:], ones1[:], pb_sb[:, 512:768],
                                     start=False, stop=True)
                nc.vector.tensor_tensor(x_sb[t][:, 0:512], x_sb[t][:, 0:512],
                                        p5[:], op=AL.add)
                nc.vector.tensor_tensor(x_sb[t][:, 512:768], x_sb[t][:, 512:768],
                                        p2[:], op=AL.add)

            # ---------------- phase 7: LN2 + h2T (branch-permuted) ----------------
            for t in range(NT):
                xh = wpool.tile([128, C], BF16, tag="xh")
                layernorm(x_sb[t], xh)
                transpose_into(hT, H2T_PERM[t] * 128, xh)

            # ---------------- phase 8+9: MLP per branch ----------------
            for br in range(2):
                # fc1 + gelu -> gT (reuses the qkT slot; qkT is dead by now)
                gT = rpool.tile([128, OC, 512], BF16, tag="qkg", name=f"gT{br}")
                for oc in range(OC):
                    slab = wtpool.tile([128, CC, 128], BF16, tag="fcw1",
                                       name=f"fc1s_{br}_{oc}")
                    nc.sync.dma_start(slab[:], fc1w_d[br, oc])
                    pt = ps.tile([128, 512], F32, tag="st")
                    for cc in range(CC):
                        nc.tensor.matmul(pt[:], slab[:, cc, :],
                                         hT[:, cc, br * 512:(br + 1) * 512],
                                         start=(cc == 0), stop=(cc == CC - 1))
                    sg = wpool.tile([128, 512], BF16, tag="sg")
                    if flags["use_fc1bias"]:
                        nc.scalar.activation(sg[:], pt[:], AF.Sigmoid,
                                             bias=bfc1s_sb[:, br, oc:oc + 1],
                                             scale=GELU_A)
                        nc.vector.scalar_tensor_tensor(
                            gT[:, oc, :], pt[:],
                            bfc1_sb[:, br, oc:oc + 1], sg[:],
                            op0=AL.add, op1=AL.mult)
                    else:
                        nc.scalar.activation(sg[:], pt[:], AF.Sigmoid,
                                             scale=GELU_A)
                        nc.vector.tensor_tensor(gT[:, oc, :], pt[:], sg[:],
                                                op=AL.mult)

                # fc2 + residual + store
                for tp in range(2):          # token-chunk pairs
                    tcs = [2 * tp, 2 * tp + 1]
                    p5s, p2s = {}, {}
                    for tc in tcs:
                        p5s[tc] = ps.tile([128, 512], F32, tag="st",
                                          name=f"fc2p5_{br}_{tc}")
                        p2s[tc] = ps2.tile([128, 256], F32, tag="st256",
                                           name=f"fc2p2_{br}_{tc}")
                    nsteps = OC + (1 if flags["use_fc2bias"] else 0)
                    for oc in range(OC):
                        slab = wtpool.tile([128, C], BF16, tag="fcw2",
                                           name=f"fc2s_{br}_{tp}_{oc}")
                        nc.sync.dma_start(slab[:], fc2w_d[br, oc])
                        for tc in tcs:
                            nc.tensor.matmul(
                                p5s[tc][:], gT[:, oc, tc * 128:(tc + 1) * 128],
                                slab[:, 0:512],
                                start=(oc == 0), stop=(oc == nsteps - 1))
                            nc.tensor.matmul(
                                p2s[tc][:], gT[:, oc, tc * 128:(tc + 1) * 128],
                                slab[:, 512:768],
                                start=(oc == 0), stop=(oc == nsteps - 1))
                    for tc in tcs:
                        if flags["use_fc2bias"]:
                            nc.tensor.matmul(p5s[tc][:], ones1b[:],
                                             bfc2_sb[:, br, 0:512],
                                             start=False, stop=True)
                            nc.tensor.matmul(p2s[tc][:], ones1b[:],
                                             bfc2_sb[:, br, 512:768],
                                             start=False, stop=True)
                        ti = FC2_TILES[br][tc]
                        nc.vector.tensor_tensor(x_sb[ti][:, 0:512],
                                                x_sb[ti][:, 0:512], p5s[tc][:],
                                                op=AL.add)
                        nc.vector.tensor_tensor(x_sb[ti][:, 512:768],
                                                x_sb[ti][:, 512:768], p2s[tc][:],
                                                op=AL.add)
                        nc.sync.dma_start(out_d[ti * 128:(ti + 1) * 128, :],
                                          x_sb[ti][:])

    nc.compile()
    return nc


def make_in_maps(shared, per_core):
    per_core_x, per_core_expb = per_core
    maps = []
    for c in range(NCORES):
        m = dict(shared)
        m["x"] = per_core_x[c]
        if per_core_expb is not None:
            m["expb"] = per_core_expb[c]
        maps.append(m)
    return maps


def run(inputs, trace=False):
    shared, per_core, flags = prep_inputs(**inputs)
    nc = build_program(flags)
    res = run_bass_kernel_spmd(nc, make_in_maps(shared, per_core),
                               core_ids=list(range(NCORES)), trace=trace)
    outs = [res.results[c]["out"].reshape(BL, N, C) for c in range(NCORES)]
    full = np.concatenate(outs, axis=0).astype(np.float32)
    return full, res


def kernel(**inputs):
    full, _ = run(inputs, trace=False)
    return full


# revision 14
# speedup vs baseline: 1.0071x; 1.0071x over previous
"""Trainium2 Bass kernel for nn_Block_84258668413379 (dense transformer block).

Strategy: data-parallel over batch B=16 across 8 NeuronCores (2 batches/core),
no collectives.  All matmuls in bf16 (f32 PSUM accumulation); layernorm and
softmax statistics in f32.  Transposed dataflow avoids attention-probability
transposes entirely:

  x --LN1--> xh --PE transpose--> h^T
  Q^T,K^T = Wqk'^T.T @ h^T        (output-transposed, bias fused in ACT copy)
  V       = h^T.T @ Wv'^T         (natural orientation)
  S^T     = K_h^T.T @ Q_h^T       [keys x queries]
  P^T     = exp(S^T) * exp(bias^T)   (bias exponential precomputed host-side:
                                      ACT exp reads PSUM directly, DVE multiply
                                      runs SBUF->SBUF in bf16)
  O^T     = [V_h | 1].T @ P^T     (extra column = softmax denominator)
  O_norm^T= O^T * bcast(1/s)      (GpSimd partition_broadcast + DVE mult)
  y       = O_norm^T.T @ Wp'^T ; x1 = x + y        (gamma1 folded into Wp')
  x1 --LN2--> xh2 --transpose--> h2^T (branch-permuted columns)
  g^T     = gelu(fc1'^T.T @ h2^T)  (sigmoid-form gelu: ACT sigmoid + DVE fuse)
  y2      = g^T.T @ fc2'^T ; out = x1 + y2         (gamma2 folded into fc2')

All LN affine params and additive biases are folded into weights host-side;
bias inputs that are all-zero emit no device instructions.
"""

import numpy as np

import concourse.bass as bass
import concourse.bacc as bacc
import concourse.mybir as mybir
from concourse import tile
from concourse.bass_utils import run_bass_kernel_spmd

# Problem constants (hardcoded per spec)
B, N, C, H = 16, 512, 768, 12
D = C // H                      # 64
SPLIT = 256
TABLE = 1024
HID = 4 * C                     # 3072
SCALE = D ** -0.5
EPS = 1e-5
NCORES = 8
BL = B // NCORES                # 2 batches per core
T = BL * N                      # 1024 tokens per core
NT = T // 128                   # 8 token tiles
CC = C // 128                   # 6 contraction chunks
OC = HID // 128                 # 24 hidden chunks
GELU_A = 1.702

F32 = mybir.dt.float32
BF16 = mybir.dt.bfloat16
NP_BF16 = mybir.dt.np(BF16)

# token tile -> h2T column block (group branch-t tokens, then branch-f)
H2T_PERM = [0, 1, 4, 5, 2, 3, 6, 7]
# fc2 token-chunk (branch-major) -> x tile index
FC2_TILES = [[0, 1, 4, 5], [2, 3, 6, 7]]  # [branch][tc]


def _f32(a):
    return np.ascontiguousarray(np.asarray(a, dtype=np.float32))


def prep_inputs(x, mask, relative_position_index, W_qkv, W_proj, b_proj, rel_table,
                ln1_g, ln1_b, gamma1, gamma2,
                ln2t_g, ln2t_b, fc1t_W, fc1t_b, fc2t_W, fc2t_b,
                ln2f_g, ln2f_b, fc1f_W, fc1f_b, fc2f_W, fc2f_b):
    """Host-side folding / layout prep. Returns (shared, per_core, flags)."""
    x = _f32(x); W_qkv = _f32(W_qkv); W_proj = _f32(W_proj); b_proj = _f32(b_proj)
    rel_table = _f32(rel_table)
    ln1_g = _f32(ln1_g); ln1_b = _f32(ln1_b)
    gamma1 = _f32(gamma1); gamma2 = _f32(gamma2)
    mask = np.asarray(mask)
    rpi = np.asarray(relative_position_index).astype(np.int64)

    # ---- QK weights (ln1 gain + q-scale folded) -> wqk[p, m, cc, j] ----
    Wqk = W_qkv[:2 * C] * ln1_g[None, :]
    Wqk[:C] *= SCALE
    WqkT = Wqk.T.copy()                                   # [C, 2C]
    wqk = WqkT.reshape(CC, 128, 12, 128).transpose(1, 2, 0, 3)  # [128,12,6,128]
    wqk = np.ascontiguousarray(wqk.astype(NP_BF16))

    bias_qk = (W_qkv[:2 * C] @ ln1_b)
    bias_qk[:C] *= SCALE
    bqk = np.ascontiguousarray(bias_qk.reshape(12, 128).T.astype(np.float32))  # [128,12]

    # ---- V weights -> wv[p, cc, j] ----
    Wv = W_qkv[2 * C:] * ln1_g[None, :]
    WvT = Wv.T.copy()                                     # [C, C]
    wv = np.ascontiguousarray(
        WvT.reshape(CC, 128, C).transpose(1, 0, 2).astype(NP_BF16))  # [128,6,768]
    bias_v = W_qkv[2 * C:] @ ln1_b                        # [C]
    vb = np.ascontiguousarray(bias_v.reshape(CC, 128).T.astype(np.float32))  # [128,6]
    use_vbias = bool(np.any(bias_v != 0.0))

    # ---- proj weights (gamma1 folded) -> wp[p, cc, j] ----
    Wp = gamma1[:, None] * W_proj                         # [C, C] (out, in)
    WpT = Wp.T.copy()
    wp = np.ascontiguousarray(
        WpT.reshape(CC, 128, C).transpose(1, 0, 2).astype(NP_BF16))  # [128,6,768]
    bias_p = gamma1 * b_proj
    pb = np.ascontiguousarray(bias_p[None, :].astype(NP_BF16))       # [1, 768]
    use_pbias = bool(np.any(bias_p != 0.0))

    # ---- exp(relative position bias), S^T layout: expB[h, kc, p(key), q] ----
    bias_full = rel_table[rpi]                            # [Nq, Nk, H]
    biasT = bias_full.transpose(2, 1, 0)                  # [H, Nk, Nq]
    mask_uniform = bool(mask.all())

    def _pack_expb(bt):
        e = np.exp(bt.astype(np.float64)).astype(np.float32)
        e = e.reshape(H, 2, 2, 128, N).transpose(0, 1, 3, 2, 4)
        return np.ascontiguousarray(
            e.reshape(H, 2, 128, 2 * N).astype(NP_BF16))  # [12,2,128,1024]

    per_core_expb = None
    if mask_uniform:
        expb = _pack_expb(biasT)
    else:
        mterm = np.where(mask, 0.0, -np.inf).astype(np.float32)      # [B, Nk]
        per_core_expb = []
        for c in range(NCORES):
            packs = []
            for b in range(BL):
                bt = biasT + mterm[c * BL + b][None, :, None]
                packs.append(_pack_expb(bt))
            per_core_expb.append(np.stack(packs))         # [BL,12,4,128,512]
        expb = per_core_expb[0]

    # ---- MLP weights per branch ----
    fc1w = np.empty((2, OC, 128, CC, 128), dtype=NP_BF16)
    fc2w = np.empty((2, OC, 128, C), dtype=NP_BF16)
    bfc1 = np.zeros((128, 2, OC), dtype=np.float32)
    bfc2 = np.zeros((1, 2, C), dtype=NP_BF16)
    use_fc1bias = False
    use_fc2bias = False
    for br, (g2, b2, w1, b1, w2, bz2) in enumerate([
            (ln2t_g, ln2t_b, fc1t_W, fc1t_b, fc2t_W, fc2t_b),
            (ln2f_g, ln2f_b, fc1f_W, fc1f_b, fc2f_W, fc2f_b)]):
        g2 = _f32(g2); b2 = _f32(b2); w1 = _f32(w1); b1 = _f32(b1)
        w2 = _f32(w2); bz2 = _f32(bz2)
        W1 = w1 * g2[None, :]                             # [HID, C]
        W1T = W1.T.copy()                                 # [C, HID]
        fc1w[br] = W1T.reshape(CC, 128, OC, 128).transpose(2, 1, 0, 3).astype(NP_BF16)
        b1f = w1 @ b2 + b1                                # [HID]
        bfc1[:, br, :] = b1f.reshape(OC, 128).T
        use_fc1bias = use_fc1bias or bool(np.any(b1f != 0.0))

        W2 = gamma2[:, None] * w2                         # [C, HID]
        W2T = W2.T.copy()                                 # [HID, C]
        fc2w[br] = W2T.reshape(OC, 128, C).astype(NP_BF16)
        b2f = gamma2 * bz2
        bfc2[0, br, :] = b2f.astype(NP_BF16)
        use_fc2bias = use_fc2bias or bool(np.any(b2f != 0.0))

    shared = dict(
        wqk=wqk, wv=wv, wp=wp, fc1w=np.ascontiguousarray(fc1w),
        fc2w=np.ascontiguousarray(fc2w), expb=expb, bqk=bqk,
        ident=np.ascontiguousarray(np.eye(128, dtype=NP_BF16)),
        vb=vb, pb=pb, bfc1=np.ascontiguousarray(bfc1), bfc2=bfc2,
        bfc1s=np.ascontiguousarray(GELU_A * bfc1),
    )
    per_core_x = [np.ascontiguousarray(x[c * BL:(c + 1) * BL].reshape(T, C))
                  for c in range(NCORES)]
    flags = dict(mask_uniform=mask_uniform, use_vbias=use_vbias,
                 use_pbias=use_pbias, use_fc1bias=use_fc1bias,
                 use_fc2bias=use_fc2bias)
    return shared, (per_core_x, per_core_expb), flags


def build_program(flags):
    """Emit the per-core Bass/Tile program. Same program runs SPMD on 8 cores."""
    mask_uniform = flags["mask_uniform"]
    nc = bacc.Bacc("TRN2", target_bir_lowering=False, debug=False,
                   num_devices=NCORES)

    x_d = nc.dram_tensor("x", [T, C], F32, kind="ExternalInput")
    wqk_d = nc.dram_tensor("wqk", [128, 12, CC, 128], BF16, kind="ExternalInput")
    wv_d = nc.dram_tensor("wv", [128, CC, C], BF16, kind="ExternalInput")
    wp_d = nc.dram_tensor("wp", [128, CC, C], BF16, kind="ExternalInput")
    fc1w_d = nc.dram_tensor("fc1w", [2, OC, 128, CC, 128], BF16, kind="ExternalInput")
    fc2w_d = nc.dram_tensor("fc2w", [2, OC, 128, C], BF16, kind="ExternalInput")
    if mask_uniform:
        expb_d = nc.dram_tensor("expb", [H, 2, 128, 2 * N], BF16,
                                kind="ExternalInput")
    else:
        expb_d = nc.dram_tensor("expb", [BL, H, 2, 128, 2 * N], BF16,
                                kind="ExternalInput")
    bqk_d = nc.dram_tensor("bqk", [128, 12], F32, kind="ExternalInput")
    ident_d = nc.dram_tensor("ident", [128, 128], BF16, kind="ExternalInput")
    vb_d = nc.dram_tensor("vb", [128, CC], F32, kind="ExternalInput")
    pb_d = nc.dram_tensor("pb", [1, C], BF16, kind="ExternalInput")
    bfc1_d = nc.dram_tensor("bfc1", [128, 2, OC], F32, kind="ExternalInput")
    bfc1s_d = nc.dram_tensor("bfc1s", [128, 2, OC], F32, kind="ExternalInput")
    bfc2_d = nc.dram_tensor("bfc2", [1, 2, C], BF16, kind="ExternalInput")
    out_d = nc.dram_tensor("out", [T, C], F32, kind="ExternalOutput")

    AL = mybir.AluOpType
    AF = mybir.ActivationFunctionType

    with tile.TileContext(nc) as tc:
        with (
            tc.tile_pool(name="const", bufs=1) as cpool,
            tc.tile_pool(name="res", bufs=1) as rpool,
            tc.tile_pool(name="work", bufs=3) as wpool,
            tc.tile_pool(name="wt", bufs=3) as wtpool,
            tc.tile_pool(name="bias2", bufs=2) as bpool,
            tc.tile_pool(name="stat", bufs=4) as spool,
            tc.tile_pool(name="ps", bufs=3, space="PSUM") as ps,       # st: 3 banks
            tc.tile_pool(name="ps2", bufs=2, space="PSUM") as ps2,     # tr: 2 banks
            tc.tile_pool(name="ps3", bufs=3, space="PSUM") as ps3,     # ot: 3 banks
        ):
            # ---------------- constants ----------------
            wqk_sb = cpool.tile([128, 12, CC, 128], BF16, tag="wqk")
            nc.sync.dma_start(wqk_sb[:], wqk_d[:])
            wv_sb = cpool.tile([128, CC, C], BF16, tag="wv")
            nc.sync.dma_start(wv_sb[:], wv_d[:])
            wp_sb = cpool.tile([128, CC, C], BF16, tag="wp")
            nc.sync.dma_start(wp_sb[:], wp_d[:])
            bqk_sb = cpool.tile([128, 12], F32, tag="bqk")
            nc.sync.dma_start(bqk_sb[:], bqk_d[:])
            ident_sb = cpool.tile([128, 128], BF16, tag="ident")
            nc.sync.dma_start(ident_sb[:], ident_d[:])
            onescol = cpool.tile([128, 1], BF16, tag="onescol")
            nc.vector.memset(onescol[:], 1.0)
            eps_sb = cpool.tile([128, 1], F32, tag="eps")
            nc.vector.memset(eps_sb[:], EPS)
            if flags["use_vbias"]:
                vb_sb = cpool.tile([128, CC], F32, tag="vb")
                nc.sync.dma_start(vb_sb[:], vb_d[:])
            if flags["use_pbias"]:
                pb_sb = cpool.tile([1, C], BF16, tag="pb")
                nc.sync.dma_start(pb_sb[:], pb_d[:])
                ones1 = cpool.tile([1, 128], BF16, tag="ones1")
                nc.vector.memset(ones1[:], 1.0)
            if flags["use_fc1bias"]:
                bfc1_sb = cpool.tile([128, 2, OC], F32, tag="bfc1")
                nc.sync.dma_start(bfc1_sb[:], bfc1_d[:])
                bfc1s_sb = cpool.tile([128, 2, OC], F32, tag="bfc1s")
                nc.sync.dma_start(bfc1s_sb[:], bfc1s_d[:])
            if flags["use_fc2bias"]:
                bfc2_sb = cpool.tile([1, 2, C], BF16, tag="bfc2")
                nc.sync.dma_start(bfc2_sb[:], bfc2_d[:])
                ones1b = cpool.tile([1, 128], BF16, tag="ones1b")
                nc.vector.memset(ones1b[:], 1.0)

            # ---------------- residents ----------------
            x_sb = [rpool.tile([128, C], F32, tag=f"x{t}", name=f"x_sb{t}")
                    for t in range(NT)]
            hT = rpool.tile([128, CC, T], BF16, tag="hT")         # h1T then h2T
            qkT = rpool.tile([128, 12, T], BF16, tag="qkg")       # later reused: gT
            v65 = rpool.tile([128, NT, H, D + 1], BF16, tag="v65")
            oTn = rpool.tile([128, CC, T], BF16, tag="oTn")

            def layernorm(x_t, xh_t):
                """xh_t (bf16) = (x_t - mean)/sqrt(var+eps), stats in f32."""
                st6 = spool.tile([128, 2, 6], F32, tag="st6")
                nc.vector.bn_stats(st6[:, 0, :], x_t[:, 0:C // 2])
                nc.vector.bn_stats(st6[:, 1, :], x_t[:, C // 2:C])
                mv = spool.tile([128, 2], F32, tag="mv")
                nc.vector.bn_aggr(mv[:], st6[:])
                sd = spool.tile([128, 1], F32, tag="sd")
                nc.scalar.activation(sd[:], mv[:, 1:2], AF.Sqrt, bias=eps_sb[:])
                rs = spool.tile([128, 1], F32, tag="rs")
                nc.vector.reciprocal(rs[:], sd[:])
                nc.vector.tensor_scalar(xh_t[:], x_t[:], mv[:, 0:1], rs[:],
                                        op0=AL.subtract, op1=AL.mult)

            def transpose_into(dst, dst_col, xh_t):
                """PE-transpose xh_t [128, C] into dst[:, cc, dst_col:+128]."""
                for cc in range(CC):
                    tr = ps2.tile([128, 128], BF16, tag="tr")
                    nc.tensor.transpose(tr[:], xh_t[:, cc * 128:(cc + 1) * 128],
                                        ident_sb[:])
                    nc.vector.tensor_copy(dst[:, cc, dst_col:dst_col + 128], tr[:])

            # ---------------- phase 1+2: LN1 + h1T ----------------
            for t in range(NT):
                nc.sync.dma_start(x_sb[t][:], x_d[t * 128:(t + 1) * 128, :])
                xh = wpool.tile([128, C], BF16, tag="xh")
                layernorm(x_sb[t], xh)
                transpose_into(hT, t * 128, xh)

            # ---------------- phase 3: Q^T / K^T ----------------
            for m in range(12):
                for n in range(2):
                    pt = ps.tile([128, 512], F32, tag="st")
                    for cc in range(CC):
                        nc.tensor.matmul(
                            pt[:], wqk_sb[:, m, cc, :],
                            hT[:, cc, n * 512:(n + 1) * 512],
                            start=(cc == 0), stop=(cc == CC - 1))
                    nc.scalar.activation(qkT[:, m, n * 512:(n + 1) * 512], pt[:],
                                         AF.Identity, bias=bqk_sb[:, m:m + 1])

            # ---------------- phase 4: V ----------------
            nc.vector.memset(v65[:, :, :, D:D + 1], 1.0)   # ones column
            for t in range(NT):
                p5 = ps.tile([128, 384], F32, tag="st", name=f"v_a{t}")
                p2 = ps.tile([128, 384], F32, tag="st", name=f"v_b{t}")
                for cc in range(CC):
                    nc.tensor.matmul(p5[:], hT[:, cc, t * 128:(t + 1) * 128],
                                     wv_sb[:, cc, 0:384],
                                     start=(cc == 0), stop=(cc == CC - 1))
                for cc in range(CC):
                    nc.tensor.matmul(p2[:], hT[:, cc, t * 128:(t + 1) * 128],
                                     wv_sb[:, cc, 384:768],
                                     start=(cc == 0), stop=(cc == CC - 1))
                nc.scalar.copy(
                    v65[:, t, 0:6, 0:D],
                    p5[:].rearrange("p (h d) -> p h d", d=D))
                nc.scalar.copy(
                    v65[:, t, 6:12, 0:D],
                    p2[:].rearrange("p (h d) -> p h d", d=D))

            # ---------------- phase 5: attention ----------------
            for h in range(12):
                par = h % 2
                base = par * 64
                m_q, m_k = h // 2, 6 + h // 2
                eb_tiles = []
                if mask_uniform:
                    for kp in range(2):
                        eb = bpool.tile([128, 1024], BF16, tag=f"eb{kp}",
                                        name=f"eb_{h}_{kp}")
                        nc.sync.dma_start(eb[:], expb_d[h, kp])
                        eb_tiles.append(eb)
                for b in range(BL):
                    if not mask_uniform:
                        eb_tiles = []
                        for kp in range(2):
                            eb = bpool.tile([128, 1024], BF16, tag=f"eb{kp}",
                                            name=f"eb_{h}_{b}_{kp}")
                            nc.sync.dma_start(eb[:], expb_d[b, h, kp])
                            eb_tiles.append(eb)
                    qs = qkT[base:base + 64, m_q, b * 512:(b + 1) * 512]
                    pts = []
                    for kc in range(4):
                        st = ps.tile([128, 512], F32, tag="st",
                                     name=f"st_{h}_{b}_{kc}")
                        nc.tensor.matmul(
                            st[:],
                            qkT[base:base + 64, m_k,
                                b * 512 + kc * 128:b * 512 + (kc + 1) * 128],
                            qs)
                        pe = wpool.tile([128, 512], BF16, tag="pe")
                        nc.scalar.activation(pe[:], st[:], AF.Exp)
                        pt_t = bpool.tile([128, 512], BF16, tag=f"pT{kc}",
                                          name=f"pT_{h}_{b}_{kc}")
                        nc.vector.tensor_tensor(
                            pt_t[:], pe[:],
                            eb_tiles[kc // 2][:, (kc % 2) * 512:(kc % 2 + 1) * 512],
                            op=AL.mult)
                        pts.append(pt_t)
                    # O^T (+ softmax denominator)
                    ot = ps3.tile([128, 512], F32, tag="ot", name=f"ot_{h}_{b}")
                    if par == 0:
                        for kc in range(4):
                            nc.tensor.matmul(ot[0:65, :],
                                             v65[:, b * 4 + kc, h, 0:D + 1],
                                             pts[kc][:],
                                             start=(kc == 0), stop=(kc == 3))
                        srow = ot[64:65, :]
                        orows = ot[0:64, :]
                    else:
                        for kc in range(4):
                            nc.tensor.matmul(ot[64:128, :],
                                             v65[:, b * 4 + kc, h, 0:D],
                                             pts[kc][:],
                                             start=(kc == 0), stop=(kc == 3))
                        ss = ps2.tile([65, 512], F32, tag="tr",
                                      name=f"ss_{h}_{b}")
                        for kc in range(4):
                            nc.tensor.matmul(ss[64:65, :], onescol[:],
                                             pts[kc][:],
                                             start=(kc == 0), stop=(kc == 3))
                        srow = ss[64:65, :]
                        orows = ot[64:128, :]
                    rrec = wpool.tile([65, 512], F32, tag="rrec")
                    nc.vector.reciprocal(rrec[64:65, :], srow)
                    rbc = wpool.tile([128, 512], F32, tag="rbc")
                    nc.gpsimd.partition_broadcast(rbc[base:base + 64, :],
                                                  rrec[64:65, :], channels=64)
                    dst = oTn[base:base + 64, h // 2, b * 512:(b + 1) * 512]
                    nc.vector.tensor_tensor(dst, orows, rbc[base:base + 64, :],
                                            op=AL.mult)
                    if flags["use_vbias"]:
                        nc.vector.tensor_scalar_add(
                            dst, dst, vb_sb[base:base + 64, h // 2:h // 2 + 1])

            # ---------------- phase 6: proj + residual ----------------
            for t in range(NT):
                p5 = ps.tile([128, 384], F32, tag="st", name=f"pj_a{t}")
                p2 = ps.tile([128, 384], F32, tag="st", name=f"pj_b{t}")
                nsteps = CC + (1 if flags["use_pbias"] else 0)
                for cc in range(CC):
                    nc.tensor.matmul(p5[:], oTn[:, cc, t * 128:(t + 1) * 128],
                                     wp_sb[:, cc, 0:384],
                                     start=(cc == 0), stop=(cc == nsteps - 1))
                for cc in range(CC):
                    nc.tensor.matmul(p2[:], oTn[:, cc, t * 128:(t + 1) * 128],
                                     wp_sb[:, cc, 384:768],
                                     start=(cc == 0), stop=(cc == nsteps - 1))
                if flags["use_pbias"]:
                    nc.tensor.matmul(p5[:], ones1[:], pb_sb[:, 0:384],
                                     start=False, stop=True)
                    nc.tensor.matmul(p2[:], ones1[:], pb_sb[:, 384:768],
                                     start=False, stop=True)
                nc.vector.tensor_tensor(x_sb[t][:, 0:384], x_sb[t][:, 0:384],
                                        p5[:], op=AL.add)
                nc.vector.tensor_tensor(x_sb[t][:, 384:768], x_sb[t][:, 384:768],
                                        p2[:], op=AL.add)

            # ---------------- phase 7: LN2 + h2T (branch-permuted) ----------------
            for t in range(NT):
                xh = wpool.tile([128, C], BF16, tag="xh")
                layernorm(x_sb[t], xh)
                transpose_into(hT, H2T_PERM[t] * 128, xh)

            # ---------------- phase 8+9: MLP per branch ----------------
            for br in range(2):
                # fc1 + gelu -> gT (reuses the qkT slot; qkT is dead by now)
                gT = rpool.tile([128, OC, 512], BF16, tag="qkg", name=f"gT{br}")
                for oc in range(OC):
                    slab = wtpool.tile([128, CC, 128], BF16, tag="fcw1",
                                       name=f"fc1s_{br}_{oc}")
                    nc.sync.dma_start(slab[:], fc1w_d[br, oc])
                    pt = ps.tile([128, 512], F32, tag="st")
                    for cc in range(CC):
                        nc.tensor.matmul(pt[:], slab[:, cc, :],
                                         hT[:, cc, br * 512:(br + 1) * 512],
                                         start=(cc == 0), stop=(cc == CC - 1))
                    sg = wpool.tile([128, 512], BF16, tag="sg")
                    if flags["use_fc1bias"]:
                        nc.scalar.activation(sg[:], pt[:], AF.Sigmoid,
                                             bias=bfc1s_sb[:, br, oc:oc + 1],
                                             scale=GELU_A)
                        nc.vector.scalar_tensor_tensor(
                            gT[:, oc, :], pt[:],
                            bfc1_sb[:, br, oc:oc + 1], sg[:],
                            op0=AL.add, op1=AL.mult)
                    else:
                        nc.scalar.activation(sg[:], pt[:], AF.Sigmoid,
                                             scale=GELU_A)
                        nc.vector.tensor_tensor(gT[:, oc, :], pt[:], sg[:],
                                                op=AL.mult)

                # fc2 + residual + store
                for tp in range(2):          # token-chunk pairs
                    tcs = [2 * tp, 2 * tp + 1]
                    p5s, p2s = {}, {}
                    for tc in tcs:
                        p5s[tc] = ps.tile([128, 384], F32, tag="st",
                                          name=f"fc2p5_{br}_{tc}")
                        p2s[tc] = ps3.tile([128, 384], F32, tag="ot",
                                           name=f"fc2p2_{br}_{tc}")
                    nsteps = OC + (1 if flags["use_fc2bias"] else 0)
                    for oc in range(OC):
                        slab = wtpool.tile([128, C], BF16, tag="fcw2",
                                           name=f"fc2s_{br}_{tp}_{oc}")
                        nc.sync.dma_start(slab[:], fc2w_d[br, oc])
                        for tc in tcs:
                            nc.tensor.matmul(
                                p5s[tc][:], gT[:, oc, tc * 128:(tc + 1) * 128],
                                slab[:, 0:384],
                                start=(oc == 0), stop=(oc == nsteps - 1))
                            nc.tensor.matmul(
                                p2s[tc][:], gT[:, oc, tc * 128:(tc + 1) * 128],
                                slab[:, 384:768],
                                start=(oc == 0), stop=(oc == nsteps - 1))
                    for tc in tcs:
                        if flags["use_fc2bias"]:
                            nc.tensor.matmul(p5s[tc][:], ones1b[:],
                                             bfc2_sb[:, br, 0:384],
                                             start=False, stop=True)
                            nc.tensor.matmul(p2s[tc][:], ones1b[:],
                                             bfc2_sb[:, br, 384:768],
                                             start=False, stop=True)
                        ti = FC2_TILES[br][tc]
                        nc.vector.tensor_tensor(x_sb[ti][:, 0:384],
                                                x_sb[ti][:, 0:384], p5s[tc][:],
                                                op=AL.add)
                        nc.vector.tensor_tensor(x_sb[ti][:, 384:768],
                                                x_sb[ti][:, 384:768], p2s[tc][:],
                                                op=AL.add)
                        nc.sync.dma_start(out_d[ti * 128:(ti + 1) * 128, :],
                                          x_sb[ti][:])

    nc.compile()
    return nc


def make_in_maps(shared, per_core):
    per_core_x, per_core_expb = per_core
    maps = []
    for c in range(NCORES):
        m = dict(shared)
        m["x"] = per_core_x[c]
        if per_core_expb is not None:
            m["expb"] = per_core_expb[c]
        maps.append(m)
    return maps


def run(inputs, trace=False):
    shared, per_core, flags = prep_inputs(**inputs)
    nc = build_program(flags)
    res = run_bass_kernel_spmd(nc, make_in_maps(shared, per_core),
                               core_ids=list(range(NCORES)), trace=trace)
    outs = [res.results[c]["out"].reshape(BL, N, C) for c in range(NCORES)]
    full = np.concatenate(outs, axis=0).astype(np.float32)
    return full, res


def kernel(**inputs):
    full, _ = run(inputs, trace=False)
    return full
